# revision 51
# baseline (speedup 1.0000x reference)
"""MoE feed-forward (top-2 routing, E=8 experts) on 8 Trainium2 NeuronCores.

Sharding: expert-parallel. The gate (a [T,8] matmul + top-2) and the
dispatch/combine permutations are computed on the host as part of
sharding the full inputs; each core runs the dense per-expert FFN
    ybT = W2.T @ relu(W1.T @ xbT + b1) + b2
over that expert's dispatched token buffer (both top-k choices), with
W1/W2 resident in SBUF in bf16 and token tiles streamed.

Kept tokens are packed contiguously per expert (choice-0 then choice-1,
arrival order, capacity-dropped tokens excluded) into nt uniform token
tiles of width N chosen per input (uniform widths keep TensorE pacing at
the warm N/2.4+2.5ns/matmul rate; the NEFF is compiled per (nt, N) and
cached).

Device layouts (per core / expert e):
  w1  [D/128, 4, 128, FF/4] bf16  W1[e], k-tile x column-chunk (chunked so
                                  the first accumulation chain starts early)
  w2  [FF/128, 128, D]   bf16   W2[e]
  xbt [nt, D/128, 128, N] bf16  xb[e].T token tiles
  b1  [128, FF/128]      f32    per-partition bias columns
  b2  [128, D/128]       f32
  ybt [D/128, nt, 128, N] f32   output ybT tiles
"""
import math
import os

import numpy as np

import concourse.bacc as bacc
import concourse.mybir as mybir
import concourse.tile as tile
from concourse.bass_utils import run_bass_kernel_spmd

# bass_utils' trace path does `from antenv.axon_hooks import ...` which is
# absent from some images; stub it so tracing degrades instead of crashing.
try:
    import antenv.axon_hooks  # noqa: F401
except ImportError:
    import sys as _sys
    import types as _types

    import antenv as _antenv

    _m = _types.ModuleType("antenv.axon_hooks")
    _m._hook = None
    _m.set_axon_ntff_profile_hook = lambda h: setattr(_m, "_hook", h)
    _m.get_axon_ntff_profile_hook = lambda: _m._hook
    _sys.modules["antenv.axon_hooks"] = _m
    _antenv.axon_hooks = _m

B, L, D, FF, E, TOPK = 4, 2048, 1024, 4096, 8, 2
T = B * L
CAP = math.ceil(1.25 * T * TOPK / E)  # 2560
NTOK = 512                             # max token tile width (one PSUM bank, f32)
KD = D // 128                          # 8
MF = FF // 128                         # 32
MD = D // 128                          # 8

BF16 = mybir.dt.np(mybir.dt.bfloat16)

_CACHE = {}


def _nch(mf):
    """w1 column-chunk count — must match between _build and the host packer."""
    return min(16, mf)


def _build(tile_sizes, KD=KD, MF=MF, MD=MD):
    """tile_sizes: uniform token-tile widths, e.g. (272,)*8."""
    assert len(set(tile_sizes)) == 1, "token tiles must be uniform"
    D_, FF_ = KD * 128, MF * 128
    nt = len(tile_sizes)
    W = max(tile_sizes)
    nc = bacc.Bacc()
    NCH = _nch(MF)               # w1 column chunks (consecutive m-groups)
    MCH = MF // NCH
    # All inputs are laid out so each SBUF tile fills with ONE wide DMA
    # (the Sync sequencer costs ~0.75us per DMA instruction — many small
    # DMAs serialize the startup): per-partition rows are contiguous, with
    # k-chunks (contraction tiles) concatenated along the free axis.
    w1_ext = nc.declare_dram_parameter("w1", [NCH, 128, KD * MCH * 128], mybir.dt.bfloat16, isOutput=False)
    w2_ext = nc.declare_dram_parameter("w2", [128, MF * D_], mybir.dt.bfloat16, isOutput=False)
    xbt_ext = nc.declare_dram_parameter("xbt", [nt, 128, KD * W], mybir.dt.bfloat16, isOutput=False)
    b_ext = nc.declare_dram_parameter("b", [128, MF + MD], mybir.dt.float32, isOutput=False)
    ybt_ext = nc.declare_dram_parameter("ybt", [MD, nt, 128, W], mybir.dt.float32, isOutput=True)

    f32 = mybir.dt.float32
    bf16 = mybir.dt.bfloat16
    RELU = mybir.ActivationFunctionType.Relu
    COPY = mybir.ActivationFunctionType.Identity

    with tile.TileContext(nc) as tc:
        with (
            tc.tile_pool(name="wpool", bufs=1) as wpool,
            tc.tile_pool(name="xpool", bufs=2) as xpool,
            tc.tile_pool(name="hpool", bufs=1) as hpool,
            tc.tile_pool(name="opool", bufs=3) as opool,
            tc.tile_pool(name="psum", bufs=3, space="PSUM") as psum,
        ):
            # Issue order: first token tile, biases, W1 chunk 0 (gates the
            # first chain), then W1/W2 interleaved so W2 is resident well
            # before GEMM2 of tile 0 consumes it.
            xb0 = xpool.tile([128, KD * tile_sizes[0]], bf16, tag="xb", name="xb0")
            nc.sync.dma_start(out=xb0[:], in_=xbt_ext[0][:, :KD * tile_sizes[0]])
            w1_sb = [
                wpool.tile([128, KD * MCH * 128], bf16, tag=f"w1_{c}", name=f"w1_{c}")
                for c in range(NCH)
            ]
            w2_sb = wpool.tile([128, MF * D_], bf16, tag="w2")
            b_sb = wpool.tile([128, MF + MD], f32, tag="b")
            w2_quarter = (MF // 4) * D_

            def w2_load(q):
                nc.sync.dma_start(
                    out=w2_sb[:, q * w2_quarter:(q + 1) * w2_quarter],
                    in_=w2_ext[:, q * w2_quarter:(q + 1) * w2_quarter],
                )

            # w1 chunks mostly first (GEMM1 of tile 0 consumes them in order),
            # w2 quarters trailing in (needed from GEMM2 of tile 0 onward).
            nc.sync.dma_start(out=w1_sb[0][:], in_=w1_ext[0])
            if NCH > 1:
                nc.sync.dma_start(out=w1_sb[1][:], in_=w1_ext[1])
            nc.sync.dma_start(out=b_sb[:], in_=b_ext[:])
            for c in range(2, NCH - 2):
                nc.sync.dma_start(out=w1_sb[c][:], in_=w1_ext[c])
            w2_load(0)
            if NCH > 2:
                nc.sync.dma_start(out=w1_sb[NCH - 2][:], in_=w1_ext[NCH - 2])
            w2_load(1)
            if NCH > 3:
                nc.sync.dma_start(out=w1_sb[NCH - 1][:], in_=w1_ext[NCH - 1])
            w2_load(2)
            w2_load(3)
            b1_sb = b_sb[:, :MF]
            b2_sb = b_sb[:, MF:]

            for n, ts in enumerate(tile_sizes):
                if n == 0:
                    xb = xb0
                else:
                    xb = xpool.tile([128, KD * ts], bf16, tag="xb", name=f"xb{n}")
                    nc.sync.dma_start(out=xb[:], in_=xbt_ext[n][:, :KD * ts])
                xb_sb = [xb[:, k * ts:(k + 1) * ts] for k in range(KD)]
                # GEMM1: hmidT[m] = relu(sum_k W1[k,m].T @ xbT[k] + b1[m])
                hm_sb = []
                for m in range(MF):
                    pt = psum.tile([128, ts], f32, tag="ps1", space="PSUM", name="ps1")
                    c, mi = divmod(m, MCH)
                    for k in range(KD):
                        nc.tensor.matmul(
                            out=pt[:],
                            lhsT=w1_sb[c][:, (k * MCH + mi) * 128:(k * MCH + mi + 1) * 128],
                            rhs=xb_sb[k],
                            start=(k == 0),
                            stop=(k == KD - 1),
                        )
                    hm = hpool.tile([128, ts], bf16, tag=f"hm_{m}", name=f"hm_{m}")
                    nc.scalar.activation(hm[:], pt[:], RELU, bias=b1_sb[:, m:m + 1])
                    hm_sb.append(hm)
                # GEMM2: ybT[m2] = sum_k2 W2[k2,m2].T @ hmidT[k2] + b2[m2]
                def g2_chain(m2, pt2, k2_range, first, last):
                    for k2 in k2_range:
                        nc.tensor.matmul(
                            out=pt2[:],
                            lhsT=w2_sb[:, k2 * D_ + m2 * 128:k2 * D_ + (m2 + 1) * 128],
                            rhs=hm_sb[k2][:],
                            start=(k2 == first),
                            stop=(k2 == last),
                        )

                def g2_out(m2, pt2):
                    ot = opool.tile([128, ts], f32, tag="ot", name="ot")
                    nc.scalar.activation(ot[:], pt2[:], COPY, bias=b2_sb[:, m2:m2 + 1])
                    nc.sync.dma_start(out=ybt_ext[m2, n][:, :ts], in_=ot[:])

                if n == 0:
                    # Tile 0 only: quarter-major accumulation across 4 banks so
                    # the last W2 quarter (still in flight from the preload)
                    # isn't needed until much later in the phase.
                    MQ = MF // 4
                    for g0 in range(0, MD, 4):
                        m2s = list(range(g0, min(g0 + 4, MD)))
                        pts = {
                            m2: psum.tile([128, ts], f32, tag="ps2", space="PSUM",
                                          bufs=4, name=f"ps2_{m2}")
                            for m2 in m2s
                        }
                        for qi in range(4):
                            for m2 in m2s:
                                g2_chain(m2, pts[m2], range(qi * MQ, (qi + 1) * MQ),
                                         0, MF - 1)
                        for m2 in m2s:
                            g2_out(m2, pts[m2])
                else:
                    for m2 in range(MD):
                        pt2 = psum.tile([128, ts], f32, tag="ps2", space="PSUM",
                                        bufs=4, name="ps2")
                        g2_chain(m2, pt2, range(MF), 0, MF - 1)
                        g2_out(m2, pt2)

    nc.compile()
    return nc


def _route(x, gate_w, gate_b):
    """Top-2 routing identical to the reference (softmax over E, top-2,
    per-expert arrival-order positions, capacity CAP)."""
    logits = x @ gate_w.T + gate_b                       # [T, E] f32
    logits = logits.astype(np.float32)
    mx = logits.max(axis=-1, keepdims=True)
    p = np.exp(logits - mx)
    p /= p.sum(axis=-1, keepdims=True)
    rows = np.arange(T)
    idx1 = np.argmax(p, axis=-1)
    p1 = p[rows, idx1]
    pm = p.copy()
    pm[rows, idx1] = -np.inf
    idx2 = np.argmax(pm, axis=-1)
    p2 = p[rows, idx2]

    def positions(idx):
        pos = np.empty(T, np.int64)
        for e in range(E):
            m = idx == e
            pos[m] = np.arange(m.sum())
        return pos

    out = []
    for idx, prb in ((idx1, p1), (idx2, p2)):
        pos = positions(idx)
        keep = pos < CAP
        out.append((idx, pos, prb, keep))
    return out


def kernel(h, gate_w, gate_b, W1, b1, W2, b2):
    h = np.asarray(h)
    x = np.ascontiguousarray(h.reshape(T, D), dtype=np.float32)
    routing = _route(x, np.asarray(gate_w, np.float32), np.asarray(gate_b, np.float32))

    # ---- dispatch: pack kept tokens contiguously per expert ----
    # slot ranges: choice-0 tokens first (arrival order), then choice-1.
    (idx1, pos1, p1, keep1), (idx2, pos2, p2, keep2) = routing
    cnt1 = np.array([((idx1 == e) & keep1).sum() for e in range(E)])
    cnt2 = np.array([((idx2 == e) & keep2).sum() for e in range(E)])
    total = cnt1 + cnt2
    maxcnt = max(16, int(total.max()))
    # Uniform token tiles (mixed widths degrade PE pacing). Width N costs
    # ~N/2.4+2.5 ns per matmul (warm TensorE), 512 matmuls per tile, plus
    # ~0.5us of phase-transition overhead per tile.
    best = None
    for cand in range(1, 17):
        N = ((math.ceil(maxcnt / cand) + 15) // 16) * 16
        if N > NTOK:
            continue
        cost = cand * 512 * (N / 2.4 + 2.5) + cand * 500
        if best is None or cost < best[0]:
            best = (cost, cand, N)
    _, nt, N = best
    tile_sizes = (N,) * nt
    if os.environ.get("MOE_FORCE_TILES"):
        tile_sizes = tuple(int(v) for v in os.environ["MOE_FORCE_TILES"].split(","))
        assert sum(tile_sizes) >= maxcnt
    nt = len(tile_sizes)
    W = max(tile_sizes)
    tp = nt * W                                 # padded slot grid (stride W/tile)

    in_maps = []
    W1 = np.asarray(W1)
    W2 = np.asarray(W2)
    b1 = np.asarray(b1, np.float32)
    b2 = np.asarray(b2, np.float32)
    for e in range(E):
        xb = np.zeros((tp, D), np.float32)
        m1 = (idx1 == e) & keep1
        m2 = (idx2 == e) & keep2
        xb[pos1[m1]] = x[m1]
        xb[cnt1[e] + pos2[m2]] = x[m2]
        # xbt[n, p, k*W + c] = xb[n*W + c, k*128 + p]
        xbt = np.ascontiguousarray(
            xb.reshape(nt, W, KD, 128).transpose(0, 3, 2, 1).reshape(nt, 128, KD * W)
        ).astype(BF16)
        NCH = _nch(MF)
        MCH = MF // NCH
        in_maps.append({
            # w1[c, p, (k*MCH+mi)*128 + j] = W1[k*128+p, (c*MCH+mi)*128 + j]
            "w1": np.ascontiguousarray(
                W1[e].reshape(KD, 128, NCH, MCH, 128).transpose(2, 1, 0, 3, 4)
            ).reshape(NCH, 128, KD * MCH * 128).astype(BF16),
            # w2[p, k2*D + j] = W2[k2*128+p, j]
            "w2": np.ascontiguousarray(
                W2[e].reshape(MF, 128, D).transpose(1, 0, 2)
            ).reshape(128, MF * D).astype(BF16),
            "xbt": xbt,
            "b": np.ascontiguousarray(
                np.concatenate(
                    [b1[e].reshape(MF, 128).T, b2[e].reshape(MD, 128).T], axis=1
                )
            ),
        })

    if tile_sizes not in _CACHE:
        _CACHE[tile_sizes] = _build(tile_sizes)
    nc = _CACHE[tile_sizes]

    trace = os.environ.get("MOE_BASS_TRACE") == "1"
    try:
        res = run_bass_kernel_spmd(nc, in_maps, core_ids=list(range(E)), trace=trace)
    except Exception:
        # first execution of a freshly compiled NEFF occasionally faults the
        # exec unit (observed under profiling); a retry succeeds
        res = run_bass_kernel_spmd(nc, in_maps, core_ids=list(range(E)), trace=trace)
    _CACHE["last_result"] = res

    # ---- combine: gather back to token order, weight, sum choices ----
    Y = np.empty((E, tp, D), np.float32)
    for e in range(E):
        ybt = res.results[e]["ybt"]                       # [MD, nt, 128, W]
        Y[e] = ybt.transpose(1, 3, 0, 2).reshape(tp, D)   # [slot, d]
    y = np.zeros((T, D), np.float32)
    for c, (idx, pos, prb, keep) in enumerate(routing):
        slot = pos if c == 0 else cnt1[idx] + pos
        rows = Y[idx, np.minimum(slot, tp - 1)]
        y += (prb * keep).astype(np.float32)[:, None] * rows
    return y.reshape(B, L, D)


# revision 52
# speedup vs baseline: 1.0105x; 1.0105x over previous
"""MoE feed-forward (top-2 routing, E=8 experts) on 8 Trainium2 NeuronCores.

Sharding: expert-parallel. The gate (a [T,8] matmul + top-2) and the
dispatch/combine permutations are computed on the host as part of
sharding the full inputs; each core runs the dense per-expert FFN
    ybT = W2.T @ relu(W1.T @ xbT + b1) + b2
over that expert's dispatched token buffer (both top-k choices), with
W1/W2 resident in SBUF in bf16 and token tiles streamed.

Kept tokens are packed contiguously per expert (choice-0 then choice-1,
arrival order, capacity-dropped tokens excluded) into nt uniform token
tiles of width N chosen per input (uniform widths keep TensorE pacing at
the warm N/2.4+2.5ns/matmul rate; the NEFF is compiled per (nt, N) and
cached).

Device layouts (per core / expert e):
  w1  [D/128, 4, 128, FF/4] bf16  W1[e], k-tile x column-chunk (chunked so
                                  the first accumulation chain starts early)
  w2  [FF/128, 128, D]   bf16   W2[e]
  xbt [nt, D/128, 128, N] bf16  xb[e].T token tiles
  b1  [128, FF/128]      f32    per-partition bias columns
  b2  [128, D/128]       f32
  ybt [D/128, nt, 128, N] f32   output ybT tiles
"""
import math
import os

import numpy as np

import concourse.bacc as bacc
import concourse.mybir as mybir
import concourse.tile as tile
from concourse.bass_utils import run_bass_kernel_spmd

# bass_utils' trace path does `from antenv.axon_hooks import ...` which is
# absent from some images; stub it so tracing degrades instead of crashing.
try:
    import antenv.axon_hooks  # noqa: F401
except ImportError:
    import sys as _sys
    import types as _types

    import antenv as _antenv

    _m = _types.ModuleType("antenv.axon_hooks")
    _m._hook = None
    _m.set_axon_ntff_profile_hook = lambda h: setattr(_m, "_hook", h)
    _m.get_axon_ntff_profile_hook = lambda: _m._hook
    _sys.modules["antenv.axon_hooks"] = _m
    _antenv.axon_hooks = _m

B, L, D, FF, E, TOPK = 4, 2048, 1024, 4096, 8, 2
T = B * L
CAP = math.ceil(1.25 * T * TOPK / E)  # 2560
NTOK = 512                             # max token tile width (one PSUM bank, f32)
KD = D // 128                          # 8
MF = FF // 128                         # 32
MD = D // 128                          # 8

BF16 = mybir.dt.np(mybir.dt.bfloat16)

_CACHE = {}


def _nch(mf):
    """w1 column-chunk count — must match between _build and the host packer."""
    return min(16, mf)


def _build(tile_sizes, KD=KD, MF=MF, MD=MD):
    """tile_sizes: uniform token-tile widths, e.g. (272,)*8."""
    assert len(set(tile_sizes)) == 1, "token tiles must be uniform"
    D_, FF_ = KD * 128, MF * 128
    nt = len(tile_sizes)
    W = max(tile_sizes)
    nc = bacc.Bacc()
    NCH = _nch(MF)               # w1 column chunks (consecutive m-groups)
    MCH = MF // NCH
    # All inputs are laid out so each SBUF tile fills with ONE wide DMA
    # (the Sync sequencer costs ~0.75us per DMA instruction — many small
    # DMAs serialize the startup): per-partition rows are contiguous, with
    # k-chunks (contraction tiles) concatenated along the free axis.
    w1_ext = nc.declare_dram_parameter("w1", [NCH, 128, KD * MCH * 128], mybir.dt.bfloat16, isOutput=False)
    w2_ext = nc.declare_dram_parameter("w2", [128, MF * D_], mybir.dt.bfloat16, isOutput=False)
    xbt_ext = nc.declare_dram_parameter("xbt", [nt, 128, KD * W], mybir.dt.bfloat16, isOutput=False)
    b_ext = nc.declare_dram_parameter("b", [128, MF + MD], mybir.dt.float32, isOutput=False)
    ybt_ext = nc.declare_dram_parameter("ybt", [MD, nt, 128, W], mybir.dt.float32, isOutput=True)

    f32 = mybir.dt.float32
    bf16 = mybir.dt.bfloat16
    RELU = mybir.ActivationFunctionType.Relu
    COPY = mybir.ActivationFunctionType.Identity

    with tile.TileContext(nc) as tc:
        with (
            tc.tile_pool(name="wpool", bufs=1) as wpool,
            tc.tile_pool(name="xpool", bufs=2) as xpool,
            tc.tile_pool(name="hpool", bufs=1) as hpool,
            tc.tile_pool(name="opool", bufs=3) as opool,
            tc.tile_pool(name="psum", bufs=3, space="PSUM") as psum,
        ):
            # Issue order: first token tile, biases, W1 chunk 0 (gates the
            # first chain), then W1/W2 interleaved so W2 is resident well
            # before GEMM2 of tile 0 consumes it.
            xb0 = xpool.tile([128, KD * tile_sizes[0]], bf16, tag="xb", name="xb0")
            nc.sync.dma_start(out=xb0[:], in_=xbt_ext[0][:, :KD * tile_sizes[0]])
            w1_sb = [
                wpool.tile([128, KD * MCH * 128], bf16, tag=f"w1_{c}", name=f"w1_{c}")
                for c in range(NCH)
            ]
            w2_sb = wpool.tile([128, MF * D_], bf16, tag="w2")
            b_sb = wpool.tile([128, MF + MD], f32, tag="b")
            w2_quarter = (MF // 4) * D_

            def w2_load(q):
                nc.sync.dma_start(
                    out=w2_sb[:, q * w2_quarter:(q + 1) * w2_quarter],
                    in_=w2_ext[:, q * w2_quarter:(q + 1) * w2_quarter],
                )

            # w1 chunks mostly first (GEMM1 of tile 0 consumes them in order),
            # w2 quarters trailing in (needed from GEMM2 of tile 0 onward).
            nc.sync.dma_start(out=w1_sb[0][:], in_=w1_ext[0])
            if NCH > 1:
                nc.sync.dma_start(out=w1_sb[1][:], in_=w1_ext[1])
            nc.sync.dma_start(out=b_sb[:], in_=b_ext[:])
            for c in range(2, NCH - 2):
                nc.sync.dma_start(out=w1_sb[c][:], in_=w1_ext[c])
            w2_load(0)
            if NCH > 2:
                nc.sync.dma_start(out=w1_sb[NCH - 2][:], in_=w1_ext[NCH - 2])
            w2_load(1)
            if NCH > 3:
                nc.sync.dma_start(out=w1_sb[NCH - 1][:], in_=w1_ext[NCH - 1])
            w2_load(2)
            w2_load(3)
            b1_sb = b_sb[:, :MF]
            b2_sb = b_sb[:, MF:]

            for n, ts in enumerate(tile_sizes):
                if n == 0:
                    xb = xb0
                else:
                    xb = xpool.tile([128, KD * ts], bf16, tag="xb", name=f"xb{n}")
                    nc.sync.dma_start(out=xb[:], in_=xbt_ext[n][:, :KD * ts])
                xb_sb = [xb[:, k * ts:(k + 1) * ts] for k in range(KD)]
                # GEMM1: hmidT[m] = relu(sum_k W1[k,m].T @ xbT[k] + b1[m])
                hm_sb = []
                for m in range(MF):
                    pt = psum.tile([128, ts], f32, tag="ps1", space="PSUM", name="ps1")
                    c, mi = divmod(m, MCH)
                    for k in range(KD):
                        nc.tensor.matmul(
                            out=pt[:],
                            lhsT=w1_sb[c][:, (k * MCH + mi) * 128:(k * MCH + mi + 1) * 128],
                            rhs=xb_sb[k],
                            start=(k == 0),
                            stop=(k == KD - 1),
                        )
                    hm = hpool.tile([128, ts], bf16, tag=f"hm_{m}", name=f"hm_{m}")
                    nc.scalar.activation(hm[:], pt[:], RELU, bias=b1_sb[:, m:m + 1])
                    hm_sb.append(hm)
                # GEMM2: ybT[m2] = sum_k2 W2[k2,m2].T @ hmidT[k2] + b2[m2]
                for m2 in range(MD):
                    pt2 = psum.tile([128, ts], f32, tag="ps2", space="PSUM", name="ps2")
                    for k2 in range(MF):
                        nc.tensor.matmul(
                            out=pt2[:],
                            lhsT=w2_sb[:, k2 * D_ + m2 * 128:k2 * D_ + (m2 + 1) * 128],
                            rhs=hm_sb[k2][:],
                            start=(k2 == 0),
                            stop=(k2 == MF - 1),
                        )
                    ot = opool.tile([128, ts], f32, tag="ot", name="ot")
                    nc.scalar.activation(ot[:], pt2[:], COPY, bias=b2_sb[:, m2:m2 + 1])
                    nc.sync.dma_start(out=ybt_ext[m2, n][:, :ts], in_=ot[:])

    nc.compile()
    return nc


def _route(x, gate_w, gate_b):
    """Top-2 routing identical to the reference (softmax over E, top-2,
    per-expert arrival-order positions, capacity CAP)."""
    logits = x @ gate_w.T + gate_b                       # [T, E] f32
    logits = logits.astype(np.float32)
    mx = logits.max(axis=-1, keepdims=True)
    p = np.exp(logits - mx)
    p /= p.sum(axis=-1, keepdims=True)
    rows = np.arange(T)
    idx1 = np.argmax(p, axis=-1)
    p1 = p[rows, idx1]
    pm = p.copy()
    pm[rows, idx1] = -np.inf
    idx2 = np.argmax(pm, axis=-1)
    p2 = p[rows, idx2]

    def positions(idx):
        pos = np.empty(T, np.int64)
        for e in range(E):
            m = idx == e
            pos[m] = np.arange(m.sum())
        return pos

    out = []
    for idx, prb in ((idx1, p1), (idx2, p2)):
        pos = positions(idx)
        keep = pos < CAP
        out.append((idx, pos, prb, keep))
    return out


def kernel(h, gate_w, gate_b, W1, b1, W2, b2):
    h = np.asarray(h)
    x = np.ascontiguousarray(h.reshape(T, D), dtype=np.float32)
    routing = _route(x, np.asarray(gate_w, np.float32), np.asarray(gate_b, np.float32))

    # ---- dispatch: pack kept tokens contiguously per expert ----
    # slot ranges: choice-0 tokens first (arrival order), then choice-1.
    (idx1, pos1, p1, keep1), (idx2, pos2, p2, keep2) = routing
    cnt1 = np.array([((idx1 == e) & keep1).sum() for e in range(E)])
    cnt2 = np.array([((idx2 == e) & keep2).sum() for e in range(E)])
    total = cnt1 + cnt2
    maxcnt = max(16, int(total.max()))
    # Uniform token tiles (mixed widths degrade PE pacing). Width N costs
    # ~N/2.4+2.5 ns per matmul (warm TensorE), 512 matmuls per tile, plus
    # ~0.5us of phase-transition overhead per tile.
    best = None
    for cand in range(1, 17):
        N = ((math.ceil(maxcnt / cand) + 15) // 16) * 16
        if N > NTOK:
            continue
        cost = cand * 512 * (N / 2.4 + 2.5) + cand * 500
        if best is None or cost < best[0]:
            best = (cost, cand, N)
    _, nt, N = best
    tile_sizes = (N,) * nt
    if os.environ.get("MOE_FORCE_TILES"):
        tile_sizes = tuple(int(v) for v in os.environ["MOE_FORCE_TILES"].split(","))
        assert sum(tile_sizes) >= maxcnt
    nt = len(tile_sizes)
    W = max(tile_sizes)
    tp = nt * W                                 # padded slot grid (stride W/tile)

    in_maps = []
    W1 = np.asarray(W1)
    W2 = np.asarray(W2)
    b1 = np.asarray(b1, np.float32)
    b2 = np.asarray(b2, np.float32)
    for e in range(E):
        xb = np.zeros((tp, D), np.float32)
        m1 = (idx1 == e) & keep1
        m2 = (idx2 == e) & keep2
        xb[pos1[m1]] = x[m1]
        xb[cnt1[e] + pos2[m2]] = x[m2]
        # xbt[n, p, k*W + c] = xb[n*W + c, k*128 + p]
        xbt = np.ascontiguousarray(
            xb.reshape(nt, W, KD, 128).transpose(0, 3, 2, 1).reshape(nt, 128, KD * W)
        ).astype(BF16)
        NCH = _nch(MF)
        MCH = MF // NCH
        in_maps.append({
            # w1[c, p, (k*MCH+mi)*128 + j] = W1[k*128+p, (c*MCH+mi)*128 + j]
            "w1": np.ascontiguousarray(
                W1[e].reshape(KD, 128, NCH, MCH, 128).transpose(2, 1, 0, 3, 4)
            ).reshape(NCH, 128, KD * MCH * 128).astype(BF16),
            # w2[p, k2*D + j] = W2[k2*128+p, j]
            "w2": np.ascontiguousarray(
                W2[e].reshape(MF, 128, D).transpose(1, 0, 2)
            ).reshape(128, MF * D).astype(BF16),
            "xbt": xbt,
            "b": np.ascontiguousarray(
                np.concatenate(
                    [b1[e].reshape(MF, 128).T, b2[e].reshape(MD, 128).T], axis=1
                )
            ),
        })

    if tile_sizes not in _CACHE:
        _CACHE[tile_sizes] = _build(tile_sizes)
    nc = _CACHE[tile_sizes]

    trace = os.environ.get("MOE_BASS_TRACE") == "1"
    try:
        res = run_bass_kernel_spmd(nc, in_maps, core_ids=list(range(E)), trace=trace)
    except Exception:
        # first execution of a freshly compiled NEFF occasionally faults the
        # exec unit (observed under profiling); a retry succeeds
        res = run_bass_kernel_spmd(nc, in_maps, core_ids=list(range(E)), trace=trace)
    _CACHE["last_result"] = res

    # ---- combine: gather back to token order, weight, sum choices ----
    Y = np.empty((E, tp, D), np.float32)
    for e in range(E):
        ybt = res.results[e]["ybt"]                       # [MD, nt, 128, W]
        Y[e] = ybt.transpose(1, 3, 0, 2).reshape(tp, D)   # [slot, d]
    y = np.zeros((T, D), np.float32)
    for c, (idx, pos, prb, keep) in enumerate(routing):
        slot = pos if c == 0 else cnt1[idx] + pos
        rows = Y[idx, np.minimum(slot, tp - 1)]
        y += (prb * keep).astype(np.float32)[:, None] * rows
    return y.reshape(B, L, D)
